# revision 10
# baseline (speedup 1.0000x reference)
"""DynamicGraphConv Trainium2 kernel (8 NeuronCores, SPMD).

Strategy: edges are sorted by destination on the host and sharded by
destination-node range (6250 nodes per core), so each core owns a disjoint
slice of the output and no cross-core collective is needed.  Node features
are gathered per-edge on the host (x[src]) and shipped edge-aligned.

Per-core device pipeline (features-on-partition layout, 512-edge tiles):
  mm1:   h_pre^T = W1z.T @ ef^T                      (PE, PSUM [65,512])
  relu:  h'^T = relu(h_pre^T + [b1;1])               (ACT -> SBUF)
  mm2:   W^T chunk c = W2p[:,c].T @ h'^T             (PE, PSUM [128,512] x2)
         (W2p row 64 = b2, folded via the ones row of h')
  xsrep: xs^T rows replicated x16 via broadcast DMA  (bf16 SBUF [128,512] x2)
  prod:  W^T * xsrep^T  -> bf16 SBUF                 (DVE)
  mm4:   msg^T += Ired_c.T @ prod                    (PE, PSUM [16,512])
  tr:    msg^T -> bf16, DMA-transpose to edge-major  ([128, 4, 16])
  segmm: per 128-edge subtile, segment-sum over the subtile's <=64-node
         window via a one-hot seg matrix (GPSIMD is_equal build, PE matmul).
         Host packs edges so no node's run crosses a subtile boundary, so
         each node's full sum lands in exactly one subtile window.
  scatter: one dma_scatter_add of the per-node partials (unique real
         indices -> race free; empty window columns aim at a sacrificial
         row) into the zeroed node accumulator in DRAM.
  final: out[n,:] = msum[n,:] * A[n] + B[n,:]        (DVE)
         A = 1/max(cnt,1) masked, B = bias or x[n]+bias (host-prepared)
"""

import os
import sys

import numpy as np

for _p in ("/opt/trn_rl_repo",):
    if _p not in sys.path and os.path.isdir(_p):
        sys.path.insert(0, _p)

import ml_dtypes  # noqa: E402

import concourse.bass as bass  # noqa: E402
import concourse.tile as tile  # noqa: E402
from concourse import bacc  # noqa: E402
from concourse import library_config  # noqa: E402
from concourse import mybir  # noqa: E402
from concourse._compat import with_exitstack  # noqa: E402
from concourse.bass_utils import run_bass_kernel_spmd  # noqa: E402

dt = mybir.dt

N_NODES = 50000
N_EDGES = 500000
F = 16
H = 64
HP = H + 1  # hidden + b2 row
NCORES = 8
NPC = N_NODES // NCORES  # 6250 nodes per core
NPAD = 6272  # 49 * 128
SACR = NPAD - 1  # sacrificial node row
TILE = 512
SUB = 128  # edges per segment subtile
WIN = 64  # node window per subtile
ST_ALIGN = 64  # subtiles per PSUM-bank batch


def _pack_core(dloc):
    """Greedy-pack sorted local-dst edges into 128-edge subtiles such that
    no node's run crosses a subtile boundary and each subtile spans < WIN
    nodes.  Returns (pos[e] -> padded position, b[t] window bases,
    Tn{node->subtile})."""
    ne = len(dloc)
    pos = np.empty(ne, np.int64)
    b = []
    Tn = {}
    t = -1
    fill = SUB  # force new subtile at first node
    base = -1
    i = 0
    while i < ne:
        n = dloc[i]
        j = i
        while j < ne and dloc[j] == n:
            j += 1
        d = j - i
        assert d <= SUB, f"degree {d} exceeds subtile size"
        if (SUB - fill) < d or (n - base) >= WIN:
            t += 1
            fill = 0
            base = n
            b.append(n)
        Tn[n] = t
        pos[i:j] = t * SUB + fill + np.arange(d)
        fill += d
        i = j
    return pos, np.array(b, np.int64), Tn


def _host_prep(x, edge_feat, src, dst, W1, b1, W2, b2, bias):
    x = np.asarray(x, np.float32)
    edge_feat = np.asarray(edge_feat, np.float32)
    src = np.asarray(src, np.int64)
    dst = np.asarray(dst, np.int64)
    W1 = np.asarray(W1, np.float32)
    b1 = np.asarray(b1, np.float32)
    W2 = np.asarray(W2, np.float32)
    b2 = np.asarray(b2, np.float32)
    bias = np.asarray(bias, np.float32)

    order = np.argsort(dst, kind="stable")
    dst_s = dst[order]
    src_s = src[order]
    bounds = np.searchsorted(dst_s, np.arange(NCORES + 1) * NPC)

    packs = []
    st_max = 0
    for c in range(NCORES):
        lo, hi = bounds[c], bounds[c + 1]
        dloc = dst_s[lo:hi] - c * NPC
        pos, b, Tn = _pack_core(dloc)
        packs.append((pos, b, Tn))
        st_max = max(st_max, len(b))
    st = int(np.ceil(st_max / ST_ALIGN) * ST_ALIGN)
    ep = st * SUB

    # shared constants
    W1z = np.concatenate([W1, np.zeros((F, 1), np.float32)], axis=1)  # [16,65]
    biasv = np.concatenate([b1, np.ones(1, np.float32)])[:, None]  # [65,1]
    W2p = np.concatenate([W2, b2[None, :]], axis=0)  # [65,256]
    Ired = np.zeros((128, 2, F), np.float32)
    for c2 in range(2):
        for m in range(128):
            Ired[m, c2, (c2 * 128 + m) % F] = 1.0
    Ired = Ired.astype(ml_dtypes.bfloat16)
    iota64 = np.broadcast_to(
        np.arange(WIN, dtype=np.float32), (128, WIN)
    ).copy()

    cnt_all = np.bincount(dst_s, minlength=N_NODES).astype(np.float32)

    in_maps = []
    for c in range(NCORES):
        lo, hi = bounds[c], bounds[c + 1]
        ec = hi - lo
        pos, b, Tn = packs[c]
        nst = len(b)

        efT = np.zeros((F, ep), np.float32)
        xsTb = np.zeros((F, ep), ml_dtypes.bfloat16)
        efT[:, pos] = edge_feat[order[lo:hi]].T
        xsTb[:, pos] = x[src_s[lo:hi]].T.astype(ml_dtypes.bfloat16)

        dloc = dst_s[lo:hi] - c * NPC
        dlr = np.full((ep,), -1.0, np.float32)
        dlr[pos] = (dloc - b[(pos // SUB)]).astype(np.float32)
        assert dlr.max() < WIN
        dlocrel = dlr.reshape(st, SUB).T.copy()  # [128, st] f32

        idx2 = np.full((st * WIN,), SACR, np.int64)
        for n, t in Tn.items():
            idx2[t * WIN + (n - b[t])] = n
        n2 = st * WIN
        idxw = np.zeros((16, n2 // 16), np.int16)
        idxw[np.arange(n2) % 16, np.arange(n2) // 16] = idx2.astype(np.int16)
        idxw = np.tile(idxw, (8, 1))

        cnt = np.zeros((NPAD,), np.float32)
        cnt[:NPC] = cnt_all[c * NPC : (c + 1) * NPC]
        A = np.where(cnt > 0, 1.0 / np.maximum(cnt, 1.0), 0.0).astype(np.float32)
        B = np.broadcast_to(bias, (NPAD, F)).copy().astype(np.float32)
        nomsg = cnt[:NPC] == 0
        if nomsg.any():
            xn = x[c * NPC : (c + 1) * NPC]
            B[:NPC][nomsg] = xn[nomsg] + bias

        in_maps.append(
            {
                "efT": efT,
                "xsTb": xsTb,
                "dlocrel": dlocrel,
                "idx2": idxw,
                "A": A[:, None],
                "B": B,
                "W1z": W1z,
                "biasv": biasv,
                "W2p": W2p,
                "Ired": Ired,
                "iota64": iota64,
            }
        )
    return in_maps, ep, bounds


@with_exitstack
def _build_tile_kernel(ctx, tc: tile.TileContext, ep: int):
    nc = tc.nc
    st = ep // SUB
    t_efT = nc.dram_tensor("efT", [F, ep], dt.float32, kind="ExternalInput").ap()
    t_xsTb = nc.dram_tensor("xsTb", [F, ep], dt.bfloat16, kind="ExternalInput").ap()
    t_dlr = nc.dram_tensor(
        "dlocrel", [128, st], dt.float32, kind="ExternalInput"
    ).ap()
    t_idx2 = nc.dram_tensor(
        "idx2", [128, (st * WIN) // 16], dt.int16, kind="ExternalInput"
    ).ap()
    t_A = nc.dram_tensor("A", [NPAD, 1], dt.float32, kind="ExternalInput").ap()
    t_B = nc.dram_tensor("B", [NPAD, F], dt.float32, kind="ExternalInput").ap()
    t_W1z = nc.dram_tensor("W1z", [F, HP], dt.float32, kind="ExternalInput").ap()
    t_biasv = nc.dram_tensor("biasv", [HP, 1], dt.float32, kind="ExternalInput").ap()
    t_W2p = nc.dram_tensor("W2p", [HP, 256], dt.float32, kind="ExternalInput").ap()
    t_Ired = nc.dram_tensor(
        "Ired", [128, 2, F], dt.bfloat16, kind="ExternalInput"
    ).ap()
    t_iota = nc.dram_tensor(
        "iota64", [128, WIN], dt.float32, kind="ExternalInput"
    ).ap()
    t_out = nc.dram_tensor("OUT", [NPAD, F], dt.float32, kind="ExternalOutput").ap()
    nb = nc.dram_tensor("node_buf", [NPAD, 64], dt.float32).ap()

    const = ctx.enter_context(tc.tile_pool(name="const", bufs=1))
    ebuf = ctx.enter_context(tc.tile_pool(name="ebuf", bufs=3))
    hbuf = ctx.enter_context(tc.tile_pool(name="hbuf", bufs=2))
    pbuf = ctx.enter_context(tc.tile_pool(name="pbuf", bufs=4))
    mbuf = ctx.enter_context(tc.tile_pool(name="mbuf", bufs=3))
    segb = ctx.enter_context(tc.tile_pool(name="segb", bufs=6))
    slab2 = ctx.enter_context(tc.tile_pool(name="slab2", bufs=1))
    fin = ctx.enter_context(tc.tile_pool(name="fin", bufs=3))
    ps_h = ctx.enter_context(tc.tile_pool(name="ps_h", bufs=2, space="PSUM"))
    ps_w = ctx.enter_context(tc.tile_pool(name="ps_w", bufs=2, space="PSUM"))
    ps_m = ctx.enter_context(tc.tile_pool(name="ps_m", bufs=2, space="PSUM"))
    ps_s = ctx.enter_context(tc.tile_pool(name="ps_s", bufs=2, space="PSUM"))

    nc.gpsimd.load_library(library_config.mlp)

    # constants
    w1z = const.tile([F, HP], dt.float32)
    nc.sync.dma_start(w1z[:], t_W1z)
    bv = const.tile([HP, 1], dt.float32)
    nc.sync.dma_start(bv[:], t_biasv)
    w2p = const.tile([HP, 256], dt.float32)
    nc.sync.dma_start(w2p[:], t_W2p)
    ired = const.tile([128, 2, F], dt.bfloat16)
    nc.sync.dma_start(ired[:], t_Ired)
    iota = const.tile([128, WIN], dt.float32)
    nc.sync.dma_start(iota[:], t_iota)
    dlrt = const.tile([128, st], dt.float32)
    nc.sync.dma_start(dlrt[:], t_dlr)
    idxt = const.tile([128, (st * WIN) // 16], dt.int16)
    nc.sync.dma_start(idxt[:], t_idx2)
    sl2 = slab2.tile([128, st // 2, F], dt.float32)

    # zero the node accumulator
    zt = const.tile([128, 64], dt.float32)
    nc.vector.memset(zt[:], 0.0)
    for t in range(NPAD // 128):
        nc.sync.dma_start(nb[t * 128 : (t + 1) * 128, :], zt[:])

    nt = ep // TILE
    for t in range(nt):
        s = slice(t * TILE, (t + 1) * TILE)
        eft = ebuf.tile([F, TILE], dt.float32, tag="eft")
        nc.sync.dma_start(eft[:], t_efT[:, s])
        xsb = ebuf.tile([F, TILE], dt.bfloat16, tag="xsb")
        nc.sync.dma_start(xsb[:], t_xsTb[:, s])

        hpre = ps_h.tile([HP, TILE], dt.float32)
        nc.tensor.matmul(hpre[:], w1z[:], eft[:], start=True, stop=True)
        hp = hbuf.tile([HP, TILE], dt.float32, tag="hp")
        nc.scalar.activation(
            hp[:], hpre[:], mybir.ActivationFunctionType.Relu, bias=bv[:]
        )

        mg = ps_m.tile([F, TILE], dt.float32, tag="mg")
        for c in range(2):
            wt = ps_w.tile([128, TILE], dt.float32, tag="wt")
            nc.tensor.matmul(
                wt[:], w2p[:, c * 128 : (c + 1) * 128], hp[:], start=True, stop=True
            )
            xr = pbuf.tile([128, TILE], dt.bfloat16, tag="xr")
            nc.sync.dma_start(
                xr[:],
                xsb[c * 8 : (c + 1) * 8, :]
                .rearrange("p (one e) -> p one e", one=1)
                .broadcast_to([8, 16, TILE]),
            )
            pr = pbuf.tile([128, TILE], dt.bfloat16, tag="prod")
            nc.vector.tensor_tensor(pr[:], wt[:], xr[:], mybir.AluOpType.mult)
            nc.tensor.matmul(
                mg[:], ired[:, c, :], pr[:], start=(c == 0), stop=(c == 1)
            )

        mt = mbuf.tile([F, TILE], dt.bfloat16, tag="msgT")
        nc.scalar.activation(mt[:], mg[:], mybir.ActivationFunctionType.Copy)
        tr = mbuf.tile([128, TILE // SUB, F], dt.bfloat16, tag="tr")
        nc.sync.dma_start(tr[:], mt[:], transpose=True)

        for q in range(TILE // SUB):
            stile = t * (TILE // SUB) + q
            seg = segb.tile([128, WIN], dt.bfloat16, tag="seg")
            nc.gpsimd.tensor_scalar(
                seg[:],
                iota[:],
                dlrt[:, stile : stile + 1],
                None,
                mybir.AluOpType.is_equal,
            )
            if stile % ST_ALIGN == 0:
                bank = ps_s.tile([128, 512], dt.float32, tag="bank")
            prow = 64 * (stile % 2)
            pcol = ((stile // 2) % (ST_ALIGN // 2)) * F
            nc.tensor.matmul(
                bank[prow : prow + WIN, pcol : pcol + F],
                seg[:],
                tr[:, q, :],
                start=True,
                stop=True,
            )
            if stile % ST_ALIGN == ST_ALIGN - 1:
                g = stile // ST_ALIGN
                nc.vector.tensor_copy(
                    sl2[:, g * (ST_ALIGN // 2) : (g + 1) * (ST_ALIGN // 2), :],
                    bank[:].rearrange("p (a b) -> p a b", b=F),
                )

    # scatter in ring-sized chunks (disjoint real rows -> safe unordered)
    csz = 4096
    for o in range(0, st * WIN, csz):
        n_i = min(csz, st * WIN - o)
        nc.gpsimd.dma_scatter_add(
            nb[:, 0:F],
            sl2[:, o // 128 : (o + n_i) // 128, :],
            idxt[:, o // 16 : (o + n_i) // 16],
            n_i,
            n_i,
            F,
            elem_step=64,
        )

    # finalize: out = msum * A + B
    for n in range(NPAD // 128):
        r = slice(n * 128, (n + 1) * 128)
        nbt = fin.tile([128, F], dt.float32, tag="nbt")
        nc.sync.dma_start(nbt[:], nb[r, 0:F])
        at = fin.tile([128, 1], dt.float32, tag="at")
        nc.sync.dma_start(at[:], t_A[r])
        bt = fin.tile([128, F], dt.float32, tag="bt")
        nc.sync.dma_start(bt[:], t_B[r])
        ot = fin.tile([128, F], dt.float32, tag="ot")
        nc.vector.scalar_tensor_tensor(
            ot[:], nbt[:], at[:], bt[:], mybir.AluOpType.mult, mybir.AluOpType.add
        )
        nc.sync.dma_start(t_out[r], ot[:])


_CACHE = {}


def _get_program(ep: int):
    if ep not in _CACHE:
        nc = bacc.Bacc("TRN2", target_bir_lowering=False, debug=False)
        with tile.TileContext(nc) as tc:
            _build_tile_kernel(tc, ep)
        nc.compile()
        _CACHE[ep] = nc
    return _CACHE[ep]


LAST_RESULTS = None


def kernel(x, edge_feat, src, dst, W1, b1, W2, b2, bias):
    global LAST_RESULTS
    in_maps, ep, _ = _host_prep(x, edge_feat, src, dst, W1, b1, W2, b2, bias)
    nc = _get_program(ep)
    trace = os.environ.get("BASS_GNN_TRACE", "") == "1"
    res = run_bass_kernel_spmd(nc, in_maps, list(range(NCORES)), trace=trace)
    LAST_RESULTS = res
    out = np.empty((N_NODES, F), np.float32)
    for c in range(NCORES):
        out[c * NPC : (c + 1) * NPC] = res.results[c]["OUT"][:NPC]
    return out


# revision 25
# speedup vs baseline: 3146.6719x; 3146.6719x over previous
"""DynamicGraphConv Trainium2 kernel (8 NeuronCores, SPMD).

Strategy: edges are sorted by destination on the host and sharded by
destination-node range (6250 nodes per core), so each core owns a disjoint
slice of the output and no cross-core collective is needed.  Node features
are gathered per-edge on the host (x[src]) and shipped edge-aligned.

Per-core device pipeline (features-on-partition layout, 512-edge tiles):
  mm1:   h_pre^T = W1z.T @ ef^T                      (PE, PSUM [65,512])
  relu:  h'^T = relu(h_pre^T + [b1;1])               (ACT -> SBUF)
  mm2:   W^T chunk c = W2p[:,c].T @ h'^T             (PE, PSUM [128,512] x2)
         (W2p row 64 = b2, folded via the ones row of h')
  xsrep: xs^T rows replicated x16 via broadcast DMA  (bf16 SBUF [128,512] x2)
  prod:  W^T * xsrep^T  -> bf16 SBUF                 (DVE)
  mm4:   msg^T += Ired_c.T @ prod                    (PE, PSUM [16,512])
  tr:    msg^T -> bf16, DMA-transpose to edge-major  ([128, 4, 16])
  segmm: per 128-edge subtile, segment-sum over the subtile's <=64-node
         window via a one-hot seg matrix (GPSIMD is_equal build, PE matmul).
         Host packs edges so no node's run crosses a subtile boundary, so
         each node's full sum lands in exactly one subtile window.
  scatter: one dma_scatter_add of the per-node partials (unique real
         indices -> race free; empty window columns aim at a sacrificial
         row) into the zeroed node accumulator in DRAM.
  final: out[n,:] = msum[n,:] * A[n] + B[n,:]        (DVE)
         A = 1/max(cnt,1) masked, B = bias or x[n]+bias (host-prepared)
"""

import os
import sys

import numpy as np

for _p in ("/opt/trn_rl_repo",):
    if _p not in sys.path and os.path.isdir(_p):
        sys.path.insert(0, _p)

import ml_dtypes  # noqa: E402

import concourse.bass as bass  # noqa: E402
import concourse.tile as tile  # noqa: E402
from concourse import bacc  # noqa: E402
from concourse import library_config  # noqa: E402
from concourse import mybir  # noqa: E402
from concourse._compat import with_exitstack  # noqa: E402
from concourse.bass_utils import run_bass_kernel_spmd  # noqa: E402

dt = mybir.dt

N_NODES = 50000
N_EDGES = 500000
F = 16
H = 64
HP = H + 1  # hidden + b2 row
NCORES = 8
NPC = N_NODES // NCORES  # 6250 nodes per core
NPAD = 6272  # 49 * 128
SACR = NPAD - 1  # sacrificial node row
TILE = 512
SUB = 128  # edges per segment subtile
WIN = 32  # node window per subtile
ST_ALIGN = 64  # subtiles per PSUM-bank batch


def _pack_core(dloc):
    """Greedy-pack sorted local-dst edges into 128-edge subtiles such that
    no node's run crosses a subtile boundary and each subtile spans < WIN
    nodes.  Returns (pos[e] -> padded position, b[t] window bases,
    Tn{node->subtile})."""
    ne = len(dloc)
    pos = np.empty(ne, np.int64)
    b = []
    Tn = {}
    t = -1
    fill = SUB  # force new subtile at first node
    base = -1
    i = 0
    while i < ne:
        n = dloc[i]
        j = i
        while j < ne and dloc[j] == n:
            j += 1
        d = j - i
        assert d <= SUB, f"degree {d} exceeds subtile size"
        if (SUB - fill) < d or (n - base) >= WIN:
            t += 1
            fill = 0
            base = n
            b.append(n)
        Tn[n] = t
        pos[i:j] = t * SUB + fill + np.arange(d)
        fill += d
        i = j
    return pos, np.array(b, np.int64), Tn


def _host_prep(x, edge_feat, src, dst, W1, b1, W2, b2, bias):
    x = np.asarray(x, np.float32)
    edge_feat = np.asarray(edge_feat, np.float32)
    src = np.asarray(src, np.int64)
    dst = np.asarray(dst, np.int64)
    W1 = np.asarray(W1, np.float32)
    b1 = np.asarray(b1, np.float32)
    W2 = np.asarray(W2, np.float32)
    b2 = np.asarray(b2, np.float32)
    bias = np.asarray(bias, np.float32)

    order = np.argsort(dst, kind="stable")
    dst_s = dst[order]
    src_s = src[order]
    bounds = np.searchsorted(dst_s, np.arange(NCORES + 1) * NPC)

    packs = []
    st_max = 0
    for c in range(NCORES):
        lo, hi = bounds[c], bounds[c + 1]
        dloc = dst_s[lo:hi] - c * NPC
        pos, b, Tn = _pack_core(dloc)
        packs.append((pos, b, Tn))
        st_max = max(st_max, len(b))
    st = int(np.ceil(st_max / ST_ALIGN) * ST_ALIGN)
    ep = st * SUB

    # shared constants
    W1z = np.concatenate([W1, np.zeros((F, 1), np.float32)], axis=1)  # [16,65]
    biasv = np.concatenate([b1, np.ones(1, np.float32)])[:, None]  # [65,1]
    W2p = np.concatenate([W2, b2[None, :]], axis=0)  # [65,256]
    Ired = np.zeros((128, 2, F), np.float32)
    for c2 in range(2):
        for m in range(128):
            Ired[m, c2, (c2 * 128 + m) % F] = 1.0
    Ired = Ired.astype(ml_dtypes.bfloat16)
    iota64 = np.broadcast_to(
        np.arange(WIN, dtype=np.float32), (128, WIN)
    ).copy()

    cnt_all = np.bincount(dst_s, minlength=N_NODES).astype(np.float32)

    in_maps = []
    for c in range(NCORES):
        lo, hi = bounds[c], bounds[c + 1]
        ec = hi - lo
        pos, b, Tn = packs[c]
        nst = len(b)

        efT = np.zeros((F, ep), np.float32)
        xsTb = np.zeros((F, ep), ml_dtypes.bfloat16)
        efT[:, pos] = edge_feat[order[lo:hi]].T
        xsTb[:, pos] = x[src_s[lo:hi]].T.astype(ml_dtypes.bfloat16)

        dloc = dst_s[lo:hi] - c * NPC
        dlr = np.full((ep,), -1.0, np.float32)
        dlr[pos] = (dloc - b[(pos // SUB)]).astype(np.float32)
        assert dlr.max() < WIN
        dlocrel = dlr.reshape(st, SUB).T.copy()  # [128, st] f32

        # scatter-entry position for (t, w): the PSUM bank/slab2 layout puts
        # subtile t at partition 64*(t%2)+w, col-group (t//2)%32, bank t//64
        idx2 = np.full((st * 64,), SACR, np.int64)
        for n, t in Tn.items():
            w = n - b[t]
            i = (t // 64) * 4096 + ((t // 2) % 32) * 128 + 64 * (t % 2) + w
            idx2[i] = n
        n2 = st * 64
        idxw = np.zeros((16, n2 // 16), np.int16)
        idxw[np.arange(n2) % 16, np.arange(n2) // 16] = idx2.astype(np.int16)
        idxw = np.tile(idxw, (8, 1))

        cnt = np.zeros((NPAD,), np.float32)
        cnt[:NPC] = cnt_all[c * NPC : (c + 1) * NPC]
        A = np.where(cnt > 0, 1.0 / np.maximum(cnt, 1.0), 0.0).astype(np.float32)
        B = np.broadcast_to(bias, (NPAD, F)).copy().astype(np.float32)
        nomsg = cnt[:NPC] == 0
        if nomsg.any():
            xn = x[c * NPC : (c + 1) * NPC]
            B[:NPC][nomsg] = xn[nomsg] + bias

        in_maps.append(
            {
                "efT": efT,
                "xsTb": xsTb,
                "dlocrel": dlocrel,
                "idx2": idxw,
                "A": A[:, None],
                "B": B,
                "W1z": W1z,
                "biasv": biasv,
                "W2p": W2p,
                "Ired": Ired,
                "iota64": iota64,
            }
        )
    return in_maps, ep, bounds


@with_exitstack
def _build_tile_kernel(ctx, tc: tile.TileContext, ep: int, parts: str = "all"):
    has = lambda p: ("all" in parts.split(",") and p in ("main","msg","seg","scat","fin")) or p in parts.split(",")
    rep = 1
    for tok in parts.split(","):
        if tok.startswith("rep"):
            rep = int(tok[3:])
    nc = tc.nc
    st = ep // SUB
    mmdt = dt.float32r if "f32r" in parts.split(",") else dt.float32
    t_efT = nc.dram_tensor("efT", [F, ep], mmdt, kind="ExternalInput").ap()
    t_xsTb = nc.dram_tensor("xsTb", [F, ep], dt.bfloat16, kind="ExternalInput").ap()
    t_dlr = nc.dram_tensor(
        "dlocrel", [128, st], dt.float32, kind="ExternalInput"
    ).ap()
    t_idx2 = nc.dram_tensor(
        "idx2", [128, (st * 64) // 16], dt.int16, kind="ExternalInput"
    ).ap()
    t_A = nc.dram_tensor("A", [NPAD, 1], dt.float32, kind="ExternalInput").ap()
    t_B = nc.dram_tensor("B", [NPAD, F], dt.float32, kind="ExternalInput").ap()
    t_W1z = nc.dram_tensor("W1z", [F, HP], mmdt, kind="ExternalInput").ap()
    t_biasv = nc.dram_tensor("biasv", [HP, 1], dt.float32, kind="ExternalInput").ap()
    t_W2p = nc.dram_tensor("W2p", [HP, 256], mmdt, kind="ExternalInput").ap()
    t_Ired = nc.dram_tensor(
        "Ired", [128, 2, F], dt.bfloat16, kind="ExternalInput"
    ).ap()
    t_iota = nc.dram_tensor(
        "iota64", [128, WIN], dt.float32, kind="ExternalInput"
    ).ap()
    t_out = nc.dram_tensor("OUT", [NPAD, F], dt.float32, kind="ExternalOutput").ap()
    nb = nc.dram_tensor("node_buf", [NPAD, 64], dt.float32).ap()

    const = ctx.enter_context(tc.tile_pool(name="const", bufs=1))
    ebuf = ctx.enter_context(tc.tile_pool(name="ebuf", bufs=3))
    hbuf = ctx.enter_context(tc.tile_pool(name="hbuf", bufs=2))
    pbuf = ctx.enter_context(tc.tile_pool(name="pbuf", bufs=4))
    mbuf = ctx.enter_context(tc.tile_pool(name="mbuf", bufs=3))
    segb = ctx.enter_context(tc.tile_pool(name="segb", bufs=6))
    slab2 = ctx.enter_context(tc.tile_pool(name="slab2", bufs=1))
    fin = ctx.enter_context(tc.tile_pool(name="fin", bufs=3))
    ps_h = ctx.enter_context(tc.tile_pool(name="ps_h", bufs=2, space="PSUM"))
    ps_w = ctx.enter_context(tc.tile_pool(name="ps_w", bufs=2, space="PSUM"))
    ps_m = ctx.enter_context(tc.tile_pool(name="ps_m", bufs=2, space="PSUM"))
    ps_s = ctx.enter_context(tc.tile_pool(name="ps_s", bufs=2, space="PSUM"))

    nc.gpsimd.load_library(library_config.mlp)

    # constants
    w1z = const.tile([F, HP], mmdt)
    nc.sync.dma_start(w1z[:], t_W1z)
    bv = const.tile([HP, 1], dt.float32)
    nc.sync.dma_start(bv[:], t_biasv)
    w2p = const.tile([HP, 256], mmdt)
    nc.sync.dma_start(w2p[:], t_W2p)
    ired = const.tile([128, 2, F], dt.bfloat16)
    nc.sync.dma_start(ired[:], t_Ired)
    iota = const.tile([128, WIN], dt.float32)
    nc.sync.dma_start(iota[:], t_iota)
    dlrt = const.tile([128, st], dt.float32)
    nc.sync.dma_start(dlrt[:], t_dlr)
    idxt = const.tile([128, (st * 64) // 16], dt.int16)
    nc.sync.dma_start(idxt[:], t_idx2)
    sl2 = slab2.tile([128, st // 2, F], dt.float32)
    nc.vector.memset(sl2[:], 0.0)

    # zero the node accumulator (batched: 1024 rows per DMA)
    JB = max(j for j in range(1, 8) if NPAD % (128 * j) == 0)
    zt = const.tile([128, JB, 64], dt.float32)
    nc.vector.memset(zt[:], 0.0)

    rep_cm = tc.For_i(0, rep, 1, name="rep") if rep > 1 else None
    if rep_cm is not None:
        ctx.enter_context(rep_cm)

    nbz = nb.rearrange("(t j p) c -> t p j c", p=128, j=JB)
    for t in range(NPAD // (128 * JB)):
        nc.sync.dma_start(nbz[t], zt[:])

    GB = 4  # tiles per DMA group
    nt = ep // TILE if has("main") else 0
    assert nt % GB == 0 or nt == 0
    for g in range(nt // GB):
        gs = slice(g * GB * TILE, (g + 1) * GB * TILE)
        eft4 = ebuf.tile([F, GB, TILE], mmdt, tag="eft")
        nc.sync.dma_start(eft4[:], t_efT[:, gs].rearrange("p (a e) -> p a e", e=TILE))
        xsb4 = ebuf.tile([F, GB * TILE], dt.bfloat16, tag="xsb")
        nc.sync.dma_start(xsb4[:], t_xsTb[:, gs])
        xr4 = [None, None]
        for c in range(2):
            xr4[c] = pbuf.tile([128, GB, TILE], dt.bfloat16, tag=f"xr{c}", name=f"xr{c}")
            nc.sync.dma_start(
                xr4[c][:].rearrange("p a e -> p (a e)"),
                xsb4[c * 8 : (c + 1) * 8, :]
                .rearrange("p (one e) -> p one e", one=1)
                .broadcast_to([8, 16, GB * TILE]),
            )
        mt4 = mbuf.tile([F, GB, TILE], dt.bfloat16, tag="msgT")

        for tt in range(GB):
            t = g * GB + tt
            hpre = ps_h.tile([HP, TILE], dt.float32)
            nc.tensor.matmul(
                hpre[:], w1z[:], eft4[:, tt, :], start=True, stop=True
            )
            hp = hbuf.tile([HP, TILE], mmdt, tag="hp")
            nc.scalar.activation(
                hp[:], hpre[:], mybir.ActivationFunctionType.Relu, bias=bv[:]
            )

            mg = ps_m.tile([F, TILE], dt.float32, tag="mg")
            for c in range(2):
                wt = ps_w.tile([128, TILE], dt.float32, tag="wt")
                nc.tensor.matmul(
                    wt[:], w2p[:, c * 128 : (c + 1) * 128], hp[:],
                    start=True, stop=True,
                )
                pr = pbuf.tile([128, TILE], dt.bfloat16, tag="prod")
                nc.vector.tensor_tensor(
                    pr[:], wt[:], xr4[c][:, tt, :], mybir.AluOpType.mult
                )
                nc.tensor.matmul(
                    mg[:], ired[:, c, :], pr[:], start=(c == 0), stop=(c == 1)
                )

            if has("msg"):
                nc.scalar.activation(
                    mt4[:, tt, :], mg[:], mybir.ActivationFunctionType.Copy
                )

        if not has("msg"):
            continue
        tr4 = mbuf.tile([128, GB * (TILE // SUB), F], dt.bfloat16, tag="tr")
        nc.sync.dma_start(
            tr4[:], mt4[:].rearrange("p a e -> p (a e)"), transpose=True
        )

        if not has("seg"):
            continue
        for q in range(GB * (TILE // SUB)):
            stile = g * GB * (TILE // SUB) + q
            seg = segb.tile([128, WIN], dt.bfloat16, tag="seg")
            nc.gpsimd.tensor_scalar(
                seg[:],
                iota[:],
                dlrt[:, stile : stile + 1],
                None,
                mybir.AluOpType.is_equal,
            )
            if stile % ST_ALIGN == 0:
                bank = ps_s.tile([128, 512], dt.float32, tag="bank")
            prow = 64 * (stile % 2)
            pcol = ((stile // 2) % (ST_ALIGN // 2)) * F
            nc.tensor.matmul(
                bank[prow : prow + WIN, pcol : pcol + F],
                seg[:],
                tr4[:, q, :],
                start=True,
                stop=True,
            )
            if stile % ST_ALIGN == ST_ALIGN - 1:
                gg = stile // ST_ALIGN
                gsl = slice(gg * (ST_ALIGN // 2), (gg + 1) * (ST_ALIGN // 2))
                for half in range(2):
                    nc.vector.tensor_copy(
                        sl2[64 * half : 64 * half + WIN, gsl, :],
                        bank[64 * half : 64 * half + WIN, :].rearrange(
                            "p (a b) -> p a b", b=F
                        ),
                    )

    # scatter in ring-sized chunks (disjoint real rows -> safe unordered)
    csz = 4096
    for o in range(0, st * 64 if has("scat") else 0, csz):
        n_i = min(csz, st * 64 - o)
        nc.gpsimd.dma_scatter_add(
            nb[:, 0:F],
            sl2[:, o // 128 : (o + n_i) // 128, :],
            idxt[:, o // 16 : (o + n_i) // 16],
            n_i,
            n_i,
            F,
            elem_step=64,
        )

    # finalize: out = msum * A + B  (batched 512 rows per step)
    FB = 4
    nfin = NPAD // 128 if has("fin") else 0
    n = 0
    while n < nfin:
        k = min(FB, nfin - n)
        r = slice(n * 128, (n + k) * 128)
        nbt = fin.tile([128, FB, F], dt.float32, tag="nbt")
        nc.sync.dma_start(
            nbt[:, :k, :], nb[r, 0:F].rearrange("(j p) c -> p j c", p=128)
        )
        at = fin.tile([128, FB, 1], dt.float32, tag="at")
        nc.sync.dma_start(at[:, :k, :], t_A[r].rearrange("(j p) c -> p j c", p=128))
        bt = fin.tile([128, FB, F], dt.float32, tag="bt")
        nc.sync.dma_start(bt[:, :k, :], t_B[r].rearrange("(j p) c -> p j c", p=128))
        ot = fin.tile([128, FB, F], dt.float32, tag="ot")
        abc = at[:, :k, :].broadcast_to([128, k, F])
        nc.vector.tensor_tensor(ot[:, :k, :], nbt[:, :k, :], abc,
                                mybir.AluOpType.mult)
        nc.vector.tensor_tensor(ot[:, :k, :], ot[:, :k, :], bt[:, :k, :],
                                mybir.AluOpType.add)
        nc.sync.dma_start(t_out[r].rearrange("(j p) c -> p j c", p=128), ot[:, :k, :])
        n += k


_CACHE = {}


PARTS = os.environ.get("BASS_GNN_PARTS", "all")


def _get_program(ep: int):
    key = (ep, PARTS)
    if key not in _CACHE:
        nc = bacc.Bacc("TRN2", target_bir_lowering=False, debug=False)
        with tile.TileContext(nc) as tc:
            _build_tile_kernel(tc, ep, parts=PARTS)
        nc.compile()
        _CACHE[key] = nc
    return _CACHE[key]


LAST_RESULTS = None


def kernel(x, edge_feat, src, dst, W1, b1, W2, b2, bias):
    global LAST_RESULTS
    in_maps, ep, _ = _host_prep(x, edge_feat, src, dst, W1, b1, W2, b2, bias)
    nc = _get_program(ep)
    trace = os.environ.get("BASS_GNN_TRACE", "") == "1"
    res = run_bass_kernel_spmd(nc, in_maps, list(range(NCORES)), trace=trace)
    LAST_RESULTS = res
    out = np.empty((N_NODES, F), np.float32)
    for c in range(NCORES):
        out[c * NPC : (c + 1) * NPC] = res.results[c]["OUT"][:NPC]
    return out
